# revision 1
# baseline (speedup 1.0000x reference)
"""Trainium2 Bass kernel for nn_AttentionLayer_68547678044407.

Per-head sigmoid-QK exp-normalized attention with length masking.

Sharding: one head per NeuronCore (8 heads / 8 cores). Every core runs an
identical program over all batches (only the weight data differs per core),
so the SPMD contract is satisfied and the load is perfectly balanced.

Sequence sparsity: each batch is padded to a multiple of 128 rows; work
scales with sum(Lp_b^2) instead of B*S^2. Rows >= seq_len never leave the
device (the host only copies the valid rows into the final output).

Math per (head h, batch b), with Lb = seq_lens[b]:
  Q^T,K^T = sigmoid(W^T x^T + b)     [64, Lp]   (bf16, stacked 128-partition)
  V'      = x W_v + b_v, ones col    [Lp, 65]   (rows >= Lb zeroed)
  S^T     = K^T.T-free matmul pairs  [128t, ns] (exp(./8) via ACT -> bf16)
  U'      = S~ @ V'                  [s, 65]    (col 64 = rowsum)
  O       = U'[:, :64] / (U'[:, 64] + 1e-8)
"""

import numpy as np

LAST_RESULT = None

import concourse.bacc as bacc
import concourse.bass as bass
import concourse.tile as tile
from concourse import mybir
from concourse.bass_utils import run_bass_kernel_spmd

H, D_IN, D_OUT = 8, 256, 64
B, S = 8, 2048
P = 128
NCORES = 8

BF16 = mybir.dt.bfloat16
FP32 = mybir.dt.float32
AF = mybir.ActivationFunctionType

_BF16_NP = mybir.dt.np(BF16)

# columns per t-chunk slot in V' / U' (65 used, padded for 8B alignment)
VC = 72
# t-chunks fused per exp activation (psum tile spans G banks)
G = 2


def _schedule(seq_lens):
    """Derive the static schedule from seq_lens (host-side)."""
    lens = [int(v) for v in seq_lens]
    chunks = [(l + P - 1) // P for l in lens]  # 128-row chunks per batch
    lp = [c * P for c in chunks]
    offs = np.concatenate([[0], np.cumsum(lp)]).astype(int)  # global row offset
    tsum = int(offs[-1])
    # query blocks per batch: (global_start, size) with size <= 512
    blocks = []
    for b in range(B):
        bb = []
        s0 = 0
        while s0 < lp[b]:
            ns = min(512, lp[b] - s0)
            bb.append((int(offs[b]) + s0, ns))
            s0 += ns
        blocks.append(bb)
    return lens, chunks, lp, offs, tsum, blocks


def _build(nc, seq_lens):
    lens, chunks, lp, offs, tsum, blocks = _schedule(seq_lens)
    nchunks = sum(chunks)

    x_t = nc.dram_tensor("xt", [2 * P, tsum], BF16, kind="ExternalInput").ap()
    wqk = nc.dram_tensor("wqk", [2, P, P], BF16, kind="ExternalInput").ap()
    wv = nc.dram_tensor("wv", [2, P, D_OUT], BF16, kind="ExternalInput").ap()
    bqk = nc.dram_tensor("bqk", [P, 1], FP32, kind="ExternalInput").ap()
    bvb = nc.dram_tensor("bvb", [P, D_OUT], FP32, kind="ExternalInput").ap()
    tmask = nc.dram_tensor("tmask", [P, B], FP32, kind="ExternalInput").ap()
    o_out = nc.dram_tensor("o", [tsum, D_OUT], FP32, kind="ExternalOutput").ap()

    with tile.TileContext(nc) as tc:
        with (
            tc.tile_pool(name="big", bufs=1) as big,
            tc.tile_pool(name="stile", bufs=3) as spool,
            tc.tile_pool(name="opool", bufs=8) as opool,
            tc.tile_pool(name="fpool", bufs=4) as fpool,
            tc.tile_pool(name="ps_s", bufs=2, space="PSUM") as ps_s,
            tc.tile_pool(name="ps_m", bufs=4, space="PSUM") as ps_m,
        ):
            # ---- persistent SBUF tensors ----
            xt_sb = big.tile([P, 2, tsum], BF16, tag="xt")
            qk_sb = big.tile([P, tsum], BF16, tag="qk")   # [q(0:64)|k(64:128), t]
            qk2_sb = big.tile([P, tsum], BF16, tag="qk2")  # swapped halves
            v_sb = big.tile([P, nchunks, VC], BF16, tag="v")
            wqk_sb = big.tile([P, 2, P], BF16, tag="wqk")
            wv_sb = big.tile([P, 2, D_OUT], BF16, tag="wv")
            bqk_sb = big.tile([P, 1], FP32, tag="bqk")
            bvb_sb = big.tile([P, D_OUT], FP32, tag="bvb")
            tm_sb = big.tile([P, B], FP32, tag="tmask")

            # small tensors first so the first projection isn't stuck behind
            # the bulk x loads in the queue
            nc.sync.dma_start(out=wqk_sb[:], in_=wqk.rearrange("c p m -> p c m"))
            nc.sync.dma_start(out=wv_sb[:], in_=wv.rearrange("c p m -> p c m"))
            nc.sync.dma_start(out=bqk_sb[:], in_=bqk)
            nc.sync.dma_start(out=bvb_sb[:], in_=bvb)
            nc.sync.dma_start(out=tm_sb[:], in_=tmask)
            for b in range(B):
                for dc in range(2):
                    nc.gpsimd.dma_start(
                        out=xt_sb[:, dc, offs[b]:offs[b] + lp[b]],
                        in_=x_t[dc * P:(dc + 1) * P, offs[b]:offs[b] + lp[b]],
                    )

            # ones column of V' (col 64 of every chunk slot)
            nc.vector.memset(v_sb[:, :, 64:65], 1.0)
            # zero source for the psum-clearing matmul
            zt_sb = big.tile([P, 512], BF16, tag="zt")
            nc.vector.memset(zt_sb[:], 0.0)

            # ---- phase A: projections ----
            for b in range(B):
                for (s0, ns) in blocks[b]:
                    pqk = ps_m.tile([P, 512], FP32, tag="m")
                    for dc in range(2):
                        nc.tensor.matmul(
                            pqk[:, :ns],
                            lhsT=wqk_sb[:, dc, :],
                            rhs=xt_sb[:, dc, s0:s0 + ns],
                            start=(dc == 0),
                            stop=(dc == 1),
                        )
                    # sigmoid(x+b) = 0.5*tanh((x+b)/2) + 0.5 — tanh shares the
                    # exp table set, so no ACT table switching ever happens
                    th = spool.tile([P, 512], BF16, tag="th")
                    nc.scalar.activation(
                        out=th[:, :ns],
                        in_=pqk[:, :ns],
                        func=AF.Tanh,
                        bias=bqk_sb[:, 0:1],
                        scale=0.5,
                    )
                    nc.vector.tensor_scalar(
                        qk_sb[:, s0:s0 + ns], th[:, :ns], 0.5, 0.5,
                        mybir.AluOpType.mult, mybir.AluOpType.add,
                    )
                    # swapped copy: K^T to partitions 0:64, Q^T to 64:128.
                    # sync queue holds only these + the small weight loads,
                    # so the copy is never stuck behind bulk traffic
                    nc.sync.dma_start(
                        out=qk2_sb[0:64, s0:s0 + ns],
                        in_=qk_sb[64:128, s0:s0 + ns],
                    )
                    nc.sync.dma_start(
                        out=qk2_sb[64:128, s0:s0 + ns],
                        in_=qk_sb[0:64, s0:s0 + ns],
                    )

                c0 = offs[b] // P
                for ci in range(chunks[b]):
                    c = c0 + ci
                    t0 = offs[b] + ci * P
                    pv = ps_m.tile([P, 512], FP32, tag="m")
                    for dc in range(2):
                        nc.tensor.matmul(
                            pv[:, :D_OUT],
                            lhsT=xt_sb[:, dc, t0:t0 + P],
                            rhs=wv_sb[:, dc, :],
                            start=(dc == 0),
                            stop=(dc == 1),
                        )
                    nc.vector.tensor_add(v_sb[:, c, 0:D_OUT], pv[:, :D_OUT], bvb_sb[:])
                # zero pad rows of the last chunk (t in [len, lp))
                if lens[b] % P != 0:
                    cl = c0 + chunks[b] - 1
                    nc.vector.tensor_scalar_mul(
                        v_sb[:, cl, 0:65], v_sb[:, cl, 0:65], tm_sb[:, b:b + 1]
                    )

            # ---- phase B: attention ----
            for b in range(B):
                c0 = offs[b] // P
                for (s0, ns) in blocks[b]:
                    nsub = ns // P
                    pu = ps_m.tile([P, 4, VC], FP32, tag="m")
                    # one K=1 matmul zeroes the accumulator bank (start=True
                    # clears has_written for the whole bank; per-j start
                    # flags would clobber sibling accumulators)
                    nc.tensor.matmul(
                        pu.rearrange("p a b -> p (a b)")[:, 0:4 * VC],
                        lhsT=zt_sb[0:1, 0:P],
                        rhs=zt_sb[0:1, 0:4 * VC],
                        start=True,
                        stop=False,
                    )
                    ngrp = (chunks[b] + G - 1) // G
                    for g in range(ngrp):
                        cg = min(G, chunks[b] - g * G)
                        pst = ps_s.tile([P, G, 512], FP32, tag="s")
                        for k in range(cg):
                            ci = g * G + k
                            t0 = offs[b] + ci * P
                            half = ci % 2
                            lhsT = (qk2_sb if half == 0 else qk_sb)[
                                half * 64:half * 64 + 64, t0:t0 + P
                            ]
                            rhs = (qk_sb if half == 0 else qk2_sb)[
                                half * 64:half * 64 + 64, s0:s0 + ns
                            ]
                            nc.tensor.matmul(
                                pst[:, k, :ns],
                                lhsT=lhsT,
                                rhs=rhs,
                                start=True,
                                stop=True,
                                tile_position=(half * 64, 0),
                            )
                        st = spool.tile([P, G, 512], BF16, tag="st")
                        nc.scalar.activation(
                            out=st[:, 0:cg, :ns],
                            in_=pst[:, 0:cg, :ns],
                            func=AF.Exp,
                            scale=0.125,
                        )
                        for k in range(cg):
                            ci = g * G + k
                            for j in range(nsub):
                                nc.tensor.matmul(
                                    pu[:, j, 0:65],
                                    lhsT=st[:, k, j * P:(j + 1) * P],
                                    rhs=v_sb[:, c0 + ci, 0:65],
                                    start=False,
                                    stop=(ci == chunks[b] - 1
                                          and j == nsub - 1),
                                )
                    ob = opool.tile([P, 4, D_OUT], FP32, tag="o")
                    for j in range(nsub):
                        r = fpool.tile([P, 1], FP32, tag="r")
                        nc.vector.tensor_scalar_add(r[:], pu[:, j, 64:65], 1e-8)
                        nc.vector.reciprocal(r[:], r[:])
                        nc.vector.tensor_scalar_mul(
                            ob[:, j, :], pu[:, j, 0:D_OUT], r[:])
                    nc.gpsimd.dma_start(
                        out=o_out[s0:s0 + ns, :].rearrange(
                            "(j p) e -> p j e", p=P),
                        in_=ob[:, 0:nsub, :],
                    )
    return nc


def kernel(**inputs):
    x = np.asarray(inputs["x_text"], dtype=np.float32)
    seq_lens = np.asarray(inputs["seq_lens"]).astype(np.int64)
    wq = np.asarray(inputs["Wq"], dtype=np.float32)
    bq = np.asarray(inputs["bq"], dtype=np.float32)
    wk = np.asarray(inputs["Wk"], dtype=np.float32)
    bk = np.asarray(inputs["bk"], dtype=np.float32)
    wv = np.asarray(inputs["Wv"], dtype=np.float32)
    bv = np.asarray(inputs["bv"], dtype=np.float32)

    lens, chunks, lp, offs, tsum, blocks = _schedule(seq_lens)

    nc = bacc.Bacc("TRN2", target_bir_lowering=False, debug=False,
                   num_devices=NCORES)
    _build(nc, seq_lens)
    nc.finalize()

    # host-side packing: x^T per batch, padded to lp[b], concatenated
    xt = np.zeros((2 * P, tsum), dtype=_BF16_NP)
    for b in range(B):
        l = lens[b]
        xt[:, offs[b]:offs[b] + l] = x[b, :l, :].T.astype(_BF16_NP)

    # per-batch tail mask: partition p valid iff p < len % 128 (for last chunk)
    tmask = np.zeros((P, B), dtype=np.float32)
    for b in range(B):
        rem = lens[b] % P
        tmask[:rem if rem else P, b] = 1.0

    in_maps = []
    for h in range(H):
        wqk = np.concatenate([wq[h], wk[h]], axis=1)  # [256, 128]
        in_maps.append({
            "xt": xt,
            "wqk": np.ascontiguousarray(
                wqk.reshape(2, P, P).astype(_BF16_NP)),
            "wv": np.ascontiguousarray(
                wv[h].reshape(2, P, D_OUT).astype(_BF16_NP)),
            # tanh-form sigmoid needs bias/2
            "bqk": (np.concatenate([bq[h], bk[h]]).reshape(P, 1) * 0.5)
                     .astype(np.float32),
            "bvb": np.broadcast_to(bv[h], (P, D_OUT)).copy().astype(np.float32),
            "tmask": tmask,
        })

    res = run_bass_kernel_spmd(nc, in_maps, list(range(NCORES)))
    global LAST_RESULT
    LAST_RESULT = res

    out = np.zeros((B, S, H * D_OUT), dtype=np.float32)
    for h in range(H):
        o = res.results[h]["o"]
        for b in range(B):
            l = lens[b]
            out[b, :l, h * D_OUT:(h + 1) * D_OUT] = o[offs[b]:offs[b] + l]
    return out

